# revision 27
# baseline (speedup 1.0000x reference)
"""Trainium2 Bass kernel for AllAtomEnergyBranch (3-layer MLP over broadcast concat).

Math (per batch b, position n, edge e):
    out[b,n,e,0] = W3^T relu(W2^T relu(Wh^T h[b,n] + We^T e_feat[e] + b1) + b2) + b3

Sharding: data-parallel over B (8 batches -> 8 NeuronCores), weights replicated.
Each core computes its [64, 256] output slice independently; no collectives.

v3 dataflow — transposed mm2 so the W3 contraction is a FREE-dim reduce:
  - |w3| is folded into W2 and b2 host-side (W2' = W2*|w3|, b2' = b2*|w3|),
    leaving only the sign vector sigma = sign(w3) for the reduce.
  - epT [512k, 256e] = (We_aug.T @ eT_aug) with b1 via augmented ones-row.
  - hpT [512k, 64n]  = Wh.T @ hT, f32.
  - per block of 2 n's (32 blocks):
      X^T[kt] [128,512] = relu(epT[kt] + hpT[kt][:,n])     (ACT, bias=hp col)
      per 128-col subtile s of the block:
        psY[s][128x,512j] = sum_kt X^T[kt][:,s].T @ W2'[kt]  (PE; X stationary,
                             W2' moving — j lives in the free dim now)
        t = psY + b2rep                                      (DVE tensor_tensor)
        z = relu(t) * sigmarep, racc[:, 4*blk+s] = sum_j z   (DVE scalar_tensor_tensor
                                                              with accum_out)
  - per iteration: PE-transpose racc [128x, 128p] -> [128p, 128x] (p = 4*blk+s
    indexes 128 consecutive outputs), ACT Identity+b3 copy to SBUF, one 64KB
    contiguous DMA ([128,128] f32 == row-major [64,256] output).

This removes the 32 serialized partition-reduce matmuls (~8us PE) and all
per-block output ops; PE does only mm2 + preamble + one transpose. The
benchmark For_i loop is 4x-unrolled with pool-rotated ep/hp/racc buffers:
fewer For_i all-engine barriers and the next iteration's preamble overlaps
the prior tail (~10us for 2x + ~4.6us more for 4x on HW; 8x regresses).

Perf (measured on TRN2 silicon, For_i-loop delta-R method; chip-thermal
variance is several us between runs):
  - TimelineSim (no LDWEIGHTS model): 122.8us single shot; PE busy 110.0us
    (= bf16 streaming roofline for mm2 + preamble + transpose), DVE ~99us,
    ACT ~65us hidden underneath.
  - hardware steady state: 135-141us/iteration depending on thermal state
    (vs 153-180us for the previous per-block-reduce kernel on the same
    days' chip states; ~28us of the HW-sim gap is per-LDWEIGHTS exposure,
    toolchain-structural at ~53ns per matmul).
  - rel err vs f32 reference: 0.0039 (harness gate 2e-2).
"""

import numpy as np
import ml_dtypes

import concourse.bass as bass
import concourse.mybir as mybir
from concourse import bacc
from concourse.bass import ts
from concourse.tile import TileContext
from concourse.bass_utils import run_bass_kernel_spmd

BF16 = mybir.dt.bfloat16
F32 = mybir.dt.float32

B, N, H = 8, 64, 256
NE, E = 256, 64
HID, OUT = 512, 1
KT = HID // 128   # 4 k-tiles of layer-1 output / layer-2 contraction
JT = HID // 128   # 4 j-tiles of layer-2 output (free dim of mm2 now)
HT = H // 128     # 2 h-tiles of layer-1 contraction
NBLK = N // 2     # blocks of 2 n-values -> 512 x-columns per block
ST = 4            # 128-col stationary subtiles per block


def build(nc, repeat=1, dyn_repeat=None, y_bufs=7, x_bufs=3, t_bufs=3,
          with_b2=False, act_x=4, unroll2=True, pt_bufs=1, max_unroll=4,
          hint_sp=False):
    """Build the per-core graph. All 8 cores run this same program.

    with_b2: include the per-free-element b2 bias add before the layer-2
    relu. When the runtime b2 vector is all zeros (as in the harness
    inputs), kernel() selects the with_b2=False graph, which skips that
    DVE pass entirely — the math is identical for b2 == 0.
    act_x: how many of the 8 X-build half-tiles per block run on ACT
    (the rest run on DVE) — engine load balancing knob.

    repeat / dyn_repeat: repeat the whole computation inside the NEFF
    (python-unrolled / For_i hardware loop) — used only for benchmarking.
    """
    ht_d = nc.declare_dram_parameter("ht", [HT, 128, N], BF16, isOutput=False)
    wh_d = nc.declare_dram_parameter("wh", [HT, 128, HID], BF16, isOutput=False)
    we_d = nc.declare_dram_parameter("we", [E + 1, HID], BF16, isOutput=False)
    et_d = nc.declare_dram_parameter("et", [E + 1, NE], BF16, isOutput=False)
    w2_d = nc.declare_dram_parameter("w2", [KT, 128, HID], BF16, isOutput=False)
    b2r_d = (nc.declare_dram_parameter("b2r", [128, HID], BF16, isOutput=False)
             if with_b2 else None)
    sgr_d = nc.declare_dram_parameter("sgr", [128, HID], BF16, isOutput=False)
    b3r_d = nc.declare_dram_parameter("b3r", [128, 1], F32, isOutput=False)
    idn_d = nc.declare_dram_parameter("idn", [128, 128], F32, isOutput=False)
    out_d = nc.declare_dram_parameter("out", [128, 128], F32, isOutput=True)

    relu = mybir.ActivationFunctionType.Relu
    ident = mybir.ActivationFunctionType.Identity
    add = mybir.AluOpType.add
    mult = mybir.AluOpType.mult
    amax = mybir.AluOpType.max

    with TileContext(nc) as tc:
        with (
            tc.tile_pool(name="const", bufs=1) as cpool,
            tc.tile_pool(name="xp", bufs=x_bufs) as xpool,
            tc.tile_pool(name="tp", bufs=t_bufs) as tpool,
            tc.tile_pool(name="zp", bufs=4) as zpool,
            tc.tile_pool(name="rp", bufs=2) as rpool,
            tc.tile_pool(name="ehp", bufs=2) as eppool,
            tc.tile_pool(name="op", bufs=2) as opool,
            tc.tile_pool(name="psY", bufs=y_bufs, space="PSUM") as y_ps,
            tc.tile_pool(name="psT", bufs=pt_bufs, space="PSUM") as t_ps,
        ):
            # ---- load weights / inputs into SBUF ----
            # Order matters: everything the preamble matmuls need (ht/we/et/wh)
            # goes first so the PE can start while W2 is still in flight.
            we_t = cpool.tile([E + 1, HID], BF16, tag="we")
            nc.sync.dma_start(out=we_t[:], in_=we_d[:])
            et_t = cpool.tile([E + 1, NE], BF16, tag="et")
            nc.sync.dma_start(out=et_t[:], in_=et_d[:])
            ht_t = []
            for h in range(HT):
                t = cpool.tile([128, N], BF16, tag=f"ht{h}", name=f"ht{h}")
                nc.sync.dma_start(out=t[:], in_=ht_d[h])
                ht_t.append(t)
            wh_t = []
            for h in range(HT):
                t = cpool.tile([128, HID], BF16, tag=f"wh{h}", name=f"wh{h}")
                nc.sync.dma_start(out=t[:], in_=wh_d[h])
                wh_t.append(t)
            b2r_t = None
            if with_b2:
                b2r_t = cpool.tile([128, HID], BF16, tag="b2r")
                nc.sync.dma_start(out=b2r_t[:], in_=b2r_d[:])
            sgr_t = cpool.tile([128, HID], BF16, tag="sgr")
            nc.sync.dma_start(out=sgr_t[:], in_=sgr_d[:])
            b3r_t = cpool.tile([128, 1], F32, tag="b3r")
            nc.sync.dma_start(out=b3r_t[:], in_=b3r_d[:])
            idn_t = cpool.tile([128, 128], F32, tag="idn")
            nc.sync.dma_start(out=idn_t[:], in_=idn_d[:])
            w2_t = []
            for k in range(KT):
                t = cpool.tile([128, HID], BF16, tag=f"w2{k}", name=f"w2{k}")
                nc.sync.dma_start(out=t[:], in_=w2_d[k])
                w2_t.append(t)

            def body():
                # ---- preamble: epT (with b1 via aug row) and hpT ----
                # ep/hp tiles come from a bufs=2 pool and are allocated inside
                # body(), so a 2x-unrolled loop double-buffers them across
                # consecutive iterations (next preamble overlaps prior tail).
                ep_t = [eppool.tile([128, NE], BF16, tag=f"ep{k}", name=f"ep{k}")
                        for k in range(KT)]
                hp_t = [eppool.tile([128, N], F32, tag=f"hp{k}", name=f"hp{k}")
                        for k in range(KT)]
                for k in range(KT):
                    ps = y_ps.tile([128, NE], F32, tag="Y", name="psE")
                    nc.tensor.matmul(
                        ps[:], we_t[:, ts(k, 128)], et_t[:], start=True, stop=True
                    )
                    nc.vector.tensor_copy(out=ep_t[k][:], in_=ps[:])
                for k in range(KT):
                    ps = y_ps.tile([128, N], F32, tag="Y", name="psH")
                    for h in range(HT):
                        nc.tensor.matmul(
                            ps[:],
                            wh_t[h][:, ts(k, 128)],
                            ht_t[h][:],
                            start=(h == 0),
                            stop=(h == HT - 1),
                        )
                    nc.vector.tensor_copy(out=hp_t[k][:], in_=ps[:])

                # per-iteration accumulator of the 128 output rows
                racc = rpool.tile([128, 128], F32, tag="racc", name="racc")

                # ---- main loop over blocks of 2 n-values ----
                for blk in range(NBLK):
                    xt = []
                    hi = 0
                    for k in range(KT):
                        t = xpool.tile([128, 512], BF16, tag=f"x{k}", name=f"x{k}")
                        for j in range(2):
                            n = 2 * blk + j
                            if hi < act_x:
                                nc.scalar.activation(
                                    out=t[:, ts(j, NE)],
                                    in_=ep_t[k][:],
                                    func=relu,
                                    bias=hp_t[k][:, n : n + 1],
                                    scale=1.0,
                                )
                            else:
                                nc.vector.tensor_scalar(
                                    out=t[:, ts(j, NE)],
                                    in0=ep_t[k][:],
                                    scalar1=hp_t[k][:, n : n + 1],
                                    scalar2=0.0,
                                    op0=add,
                                    op1=amax,
                                )
                            hi += 1
                        xt.append(t)

                    for s in range(ST):
                        psy = y_ps.tile([128, 512], F32, tag="Y")
                        for k in range(KT):
                            nc.tensor.matmul(
                                psy[:],
                                xt[k][:, ts(s, 128)],
                                w2_t[k][:],
                                start=(k == 0),
                                stop=(k == KT - 1),
                            )
                        src = psy
                        if with_b2:
                            tt = tpool.tile([128, 512], BF16, tag=f"t{s}",
                                            name=f"t{s}")
                            nc.vector.tensor_add(out=tt[:], in0=psy[:],
                                                 in1=b2r_t[:])
                            src = tt
                        zt = zpool.tile([128, 512], BF16, tag="z", name="z")
                        nc.vector.scalar_tensor_tensor(
                            out=zt[:],
                            in0=src[:],
                            scalar=0.0,
                            in1=sgr_t[:],
                            op0=amax,
                            op1=mult,
                            accum_out=racc[:, 4 * blk + s : 4 * blk + s + 1],
                        )

                # ---- epilogue: transpose racc, add b3, one contiguous DMA ----
                pst = t_ps.tile([128, 128], F32, tag="pT", name="pT")
                nc.tensor.transpose(pst[:], racc[:], idn_t[:])
                ot = opool.tile([128, 128], F32, tag="o")
                nc.scalar.activation(
                    out=ot[:], in_=pst[:], func=ident,
                    bias=b3r_t[:, 0:1], scale=1.0,
                )
                nc.sync.dma_start(out=out_d[:], in_=ot[:])

            if dyn_repeat is not None:
                hint = (mybir.EngineType.PE, mybir.EngineType.DVE,
                        mybir.EngineType.Activation)
                if hint_sp:
                    hint = hint + (mybir.EngineType.SP,)
                # Unroll the hardware loop by the largest divisor of
                # dyn_repeat <= max_unroll: fewer For_i all-engine barriers,
                # and pool rotation double-buffers ep/hp/racc across the
                # unrolled bodies. Total body count stays exactly dyn_repeat.
                u = 1
                if unroll2:
                    for cand in (8, 4, 2):
                        if cand <= max_unroll and dyn_repeat % cand == 0:
                            u = cand
                            break
                with tc.For_i(0, dyn_repeat // u, 1, hint_engines=hint):
                    for _ in range(u):
                        body()
            else:
                for _rep in range(repeat):
                    body()
    return nc


def make_in_maps(h_all, e_feat, W1, b1, W2, b2, W3, b3):
    bf = ml_dtypes.bfloat16
    Wh = np.ascontiguousarray(W1[:H]).astype(bf).reshape(HT, 128, HID)
    We_aug = np.concatenate([W1[H:], b1[None, :]], axis=0).astype(bf)
    eT_aug = np.concatenate(
        [e_feat.T, np.ones((1, NE), np.float32)], axis=0
    ).astype(bf)
    w3 = np.asarray(W3, np.float32).reshape(HID)
    aw3 = np.abs(w3)
    sg3 = np.sign(w3).astype(np.float32)
    W2s = (np.asarray(W2, np.float32) * aw3[None, :])
    b2s = np.asarray(b2, np.float32) * aw3
    W2k = W2s.astype(bf).reshape(KT, 128, HID)
    b2r = np.broadcast_to(b2s[None, :], (128, HID)).astype(bf)
    sgr = np.broadcast_to(sg3[None, :], (128, HID)).astype(bf)
    b3r = np.full((128, 1), float(np.asarray(b3).reshape(-1)[0]), np.float32)
    idn = np.eye(128, dtype=np.float32)
    shared = {
        "wh": Wh, "we": We_aug, "et": eT_aug, "w2": W2k,
        "b2r": np.ascontiguousarray(b2r), "sgr": np.ascontiguousarray(sgr),
        "b3r": b3r, "idn": idn,
    }
    in_maps = []
    for b in range(B):
        hT = np.ascontiguousarray(h_all[b].T).astype(bf).reshape(HT, 128, N)
        in_maps.append({"ht": hT, **shared})
    return in_maps


_nc_cache = {}


def _get_nc(with_b2):
    key = ("nc", bool(with_b2))
    if key not in _nc_cache:
        nc = bacc.Bacc("TRN2", target_bir_lowering=False, debug=False, num_devices=B)
        build(nc, with_b2=with_b2)
        nc.compile()
        _nc_cache[key] = nc
    return _nc_cache[key]


def kernel(h_all, e_feat, W1, b1, W2, b2, W3, b3):
    h_all = np.asarray(h_all, np.float32)
    e_feat = np.asarray(e_feat, np.float32)
    W1 = np.asarray(W1, np.float32)
    b1 = np.asarray(b1, np.float32)
    W2 = np.asarray(W2, np.float32)
    b2 = np.asarray(b2, np.float32)
    W3 = np.asarray(W3, np.float32)
    b3 = np.asarray(b3, np.float32)

    # b2 == 0 makes the pre-relu bias add a no-op; select the graph that
    # skips that pass (the general graph handles nonzero b2).
    with_b2 = bool(np.any(b2 != 0))
    nc = _get_nc(with_b2)
    in_maps = make_in_maps(h_all, e_feat, W1, b1, W2, b2, W3, b3)
    res = run_bass_kernel_spmd(nc, in_maps, core_ids=list(range(B)))
    out = np.stack([res.results[i]["out"].reshape(N, NE, OUT) for i in range(B)])
    return out.astype(np.float32)


# revision 37
# speedup vs baseline: 1.0132x; 1.0132x over previous
"""Trainium2 Bass kernel for AllAtomEnergyBranch (3-layer MLP over broadcast concat).

Math (per batch b, position n, edge e):
    out[b,n,e,0] = W3^T relu(W2^T relu(Wh^T h[b,n] + We^T e_feat[e] + b1) + b2) + b3

Sharding: data-parallel over B (8 batches -> 8 NeuronCores), weights replicated.
Each core computes its [64, 256] output slice independently; no collectives.

v3 dataflow — transposed mm2 so the W3 contraction is a FREE-dim reduce:
  - |w3| is folded into W2 and b2 host-side (W2' = W2*|w3|, b2' = b2*|w3|),
    leaving only the sign vector sigma = sign(w3) for the reduce.
  - epT [512k, 256e] = (We_aug.T @ eT_aug) with b1 via augmented ones-row.
  - hpT [512k, 64n]  = Wh.T @ hT, f32.
  - per block of 2 n's (32 blocks):
      X^T[kt] [128,512] = relu(epT[kt] + hpT[kt][:,n])     (ACT, bias=hp col)
      per 128-col subtile s of the block:
        psY[s][128x,512j] = sum_kt X^T[kt][:,s].T @ W2'[kt]  (PE; X stationary,
                             W2' moving — j lives in the free dim now)
        t = psY + b2rep                                      (DVE tensor_tensor)
        z = relu(t) * sigmarep, racc[:, 4*blk+s] = sum_j z   (DVE scalar_tensor_tensor
                                                              with accum_out)
  - per iteration: PE-transpose racc [128x, 128p] -> [128p, 128x] (p = 4*blk+s
    indexes 128 consecutive outputs), ACT Identity+b3 copy to SBUF, one 64KB
    contiguous DMA ([128,128] f32 == row-major [64,256] output).

This removes the 32 serialized partition-reduce matmuls (~8us PE) and all
per-block output ops; PE does only mm2 + preamble + one transpose. The
benchmark For_i loop is 4x-unrolled with pool-rotated ep/hp/racc buffers:
fewer For_i all-engine barriers and the next iteration's preamble overlaps
the prior tail (~10us for 2x + ~4.6us more for 4x on HW; 8x regresses,
as does adding SP to hint_engines). act_x=4 (X-build split 4 ACT / 4 DVE)
beat act_x=5 by ~3us under the 4x unroll in interleaved A/B; successive
A/B rounds walked it down to act_x=2 (each step ~2us: the DVE's 3x-faster
per-op X-build outweighs its extra load until ~act_x=2). Per-tile ACT/DVE
half-splitting measured slightly worse than block assignment. The preamble
interleaves (ep[k], hp[k]) pairs in k-order so the first X-build starts as
early as possible after the barrier (-2.1us with ehp/racc pools at bufs=4).
X-in-fp8-e3m4 (x_fp8 flag) halves LDWEIGHTS streaming and measured -3.6us
at rel err 0.0161 / absmax 0.0231 — left off by default (absmax crosses
2e-2 if the gate were interpreted that way).

Perf (measured on TRN2 silicon, For_i-loop delta-R method; chip-thermal
variance is several us between runs):
  - TimelineSim (no LDWEIGHTS model): 122.8us single shot; PE busy 110.0us
    (= bf16 streaming roofline for mm2 + preamble + transpose), DVE ~99us,
    ACT ~65us hidden underneath.
  - hardware steady state: 135-148us/iteration depending on thermal state
    (vs 153-180us for the previous per-block-reduce kernel on the same
    days' chip states; ~28us of the HW-sim gap is per-LDWEIGHTS exposure,
    toolchain-structural at ~53ns per matmul).
  - rel err vs f32 reference: 0.0039 (harness gate 2e-2).
"""

import numpy as np
import ml_dtypes

import concourse.bass as bass
import concourse.mybir as mybir
from concourse import bacc
from concourse.bass import ts
from concourse.tile import TileContext
from concourse.bass_utils import run_bass_kernel_spmd

BF16 = mybir.dt.bfloat16
F32 = mybir.dt.float32

B, N, H = 8, 64, 256
NE, E = 256, 64
HID, OUT = 512, 1
KT = HID // 128   # 4 k-tiles of layer-1 output / layer-2 contraction
JT = HID // 128   # 4 j-tiles of layer-2 output (free dim of mm2 now)
HT = H // 128     # 2 h-tiles of layer-1 contraction
NBLK = N // 2     # blocks of 2 n-values -> 512 x-columns per block
ST = 4            # 128-col stationary subtiles per block


def build(nc, repeat=1, dyn_repeat=None, y_bufs=7, x_bufs=3, t_bufs=3,
          with_b2=False, act_x=0, unroll2=True, pt_bufs=1, max_unroll=4,
          hint_sp=False, x_fp8=False, ehp_bufs=4, r_bufs=4,
          pre_ilv=True, x_split=False):
    """Build the per-core graph. All 8 cores run this same program.

    with_b2: include the per-free-element b2 bias add before the layer-2
    relu. When the runtime b2 vector is all zeros (as in the harness
    inputs), kernel() selects the with_b2=False graph, which skips that
    DVE pass entirely — the math is identical for b2 == 0.
    act_x: how many of the 8 X-build half-tiles per block run on ACT
    (the rest run on DVE) — engine load balancing knob.
    x_fp8: store X in fp8-e3m4 instead of bf16. FWL streams 4 fp8/cycle
    vs 2 bf16, halving the per-matmul LDWEIGHTS cost (~27ns vs ~53ns,
    ~13us/iter); the matmul itself runs at bf16 speed (no DoubleRow).
    Costs accuracy: rel err 0.016 vs 0.0039 (gate 2e-2) on the harness
    inputs — kernel() keeps bf16; opt-in for benchmarking.

    repeat / dyn_repeat: repeat the whole computation inside the NEFF
    (python-unrolled / For_i hardware loop) — used only for benchmarking.
    """
    ht_d = nc.declare_dram_parameter("ht", [HT, 128, N], BF16, isOutput=False)
    wh_d = nc.declare_dram_parameter("wh", [HT, 128, HID], BF16, isOutput=False)
    we_d = nc.declare_dram_parameter("we", [E + 1, HID], BF16, isOutput=False)
    et_d = nc.declare_dram_parameter("et", [E + 1, NE], BF16, isOutput=False)
    w2_d = nc.declare_dram_parameter("w2", [KT, 128, HID], BF16, isOutput=False)
    b2r_d = (nc.declare_dram_parameter("b2r", [128, HID], BF16, isOutput=False)
             if with_b2 else None)
    sgr_d = nc.declare_dram_parameter("sgr", [128, HID], BF16, isOutput=False)
    b3r_d = nc.declare_dram_parameter("b3r", [128, 1], F32, isOutput=False)
    idn_d = nc.declare_dram_parameter("idn", [128, 128], F32, isOutput=False)
    out_d = nc.declare_dram_parameter("out", [128, 128], F32, isOutput=True)

    XDT = mybir.dt.float8e3 if x_fp8 else BF16
    relu = mybir.ActivationFunctionType.Relu
    ident = mybir.ActivationFunctionType.Identity
    add = mybir.AluOpType.add
    mult = mybir.AluOpType.mult
    amax = mybir.AluOpType.max

    with TileContext(nc) as tc:
        with (
            tc.tile_pool(name="const", bufs=1) as cpool,
            tc.tile_pool(name="xp", bufs=x_bufs) as xpool,
            tc.tile_pool(name="tp", bufs=t_bufs) as tpool,
            tc.tile_pool(name="zp", bufs=4) as zpool,
            tc.tile_pool(name="rp", bufs=r_bufs) as rpool,
            tc.tile_pool(name="ehp", bufs=ehp_bufs) as eppool,
            tc.tile_pool(name="op", bufs=2) as opool,
            tc.tile_pool(name="psY", bufs=y_bufs, space="PSUM") as y_ps,
            tc.tile_pool(name="psT", bufs=pt_bufs, space="PSUM") as t_ps,
        ):
            # ---- load weights / inputs into SBUF ----
            # Order matters: everything the preamble matmuls need (ht/we/et/wh)
            # goes first so the PE can start while W2 is still in flight.
            we_t = cpool.tile([E + 1, HID], BF16, tag="we")
            nc.sync.dma_start(out=we_t[:], in_=we_d[:])
            et_t = cpool.tile([E + 1, NE], BF16, tag="et")
            nc.sync.dma_start(out=et_t[:], in_=et_d[:])
            ht_t = []
            for h in range(HT):
                t = cpool.tile([128, N], BF16, tag=f"ht{h}", name=f"ht{h}")
                nc.sync.dma_start(out=t[:], in_=ht_d[h])
                ht_t.append(t)
            wh_t = []
            for h in range(HT):
                t = cpool.tile([128, HID], BF16, tag=f"wh{h}", name=f"wh{h}")
                nc.sync.dma_start(out=t[:], in_=wh_d[h])
                wh_t.append(t)
            b2r_t = None
            if with_b2:
                b2r_t = cpool.tile([128, HID], BF16, tag="b2r")
                nc.sync.dma_start(out=b2r_t[:], in_=b2r_d[:])
            sgr_t = cpool.tile([128, HID], BF16, tag="sgr")
            nc.sync.dma_start(out=sgr_t[:], in_=sgr_d[:])
            b3r_t = cpool.tile([128, 1], F32, tag="b3r")
            nc.sync.dma_start(out=b3r_t[:], in_=b3r_d[:])
            idn_t = cpool.tile([128, 128], F32, tag="idn")
            nc.sync.dma_start(out=idn_t[:], in_=idn_d[:])
            w2_t = []
            for k in range(KT):
                t = cpool.tile([128, HID], BF16, tag=f"w2{k}", name=f"w2{k}")
                nc.sync.dma_start(out=t[:], in_=w2_d[k])
                w2_t.append(t)

            def body():
                # ---- preamble: epT (with b1 via aug row) and hpT ----
                # ep/hp tiles come from a bufs=2 pool and are allocated inside
                # body(), so a 2x-unrolled loop double-buffers them across
                # consecutive iterations (next preamble overlaps prior tail).
                ep_t = [eppool.tile([128, NE], BF16, tag=f"ep{k}", name=f"ep{k}")
                        for k in range(KT)]
                hp_t = [eppool.tile([128, N], F32, tag=f"hp{k}", name=f"hp{k}")
                        for k in range(KT)]
                def emit_psE(k):
                    ps = y_ps.tile([128, NE], F32, tag="Y", name="psE")
                    nc.tensor.matmul(
                        ps[:], we_t[:, ts(k, 128)], et_t[:], start=True, stop=True
                    )
                    nc.vector.tensor_copy(out=ep_t[k][:], in_=ps[:])

                def emit_psH(k):
                    ps = y_ps.tile([128, N], F32, tag="Y", name="psH")
                    for h in range(HT):
                        nc.tensor.matmul(
                            ps[:],
                            wh_t[h][:, ts(k, 128)],
                            ht_t[h][:],
                            start=(h == 0),
                            stop=(h == HT - 1),
                        )
                    nc.vector.tensor_copy(out=hp_t[k][:], in_=ps[:])

                if pre_ilv:
                    # (ep[k], hp[k]) pairs complete in k-order so the first
                    # X-build can start as early as possible after the
                    # loop-back barrier.
                    for k in range(KT):
                        emit_psE(k)
                        emit_psH(k)
                else:
                    for k in range(KT):
                        emit_psE(k)
                    for k in range(KT):
                        emit_psH(k)

                # per-iteration accumulator of the 128 output rows
                racc = rpool.tile([128, 128], F32, tag="racc", name="racc")

                # ---- main loop over blocks of 2 n-values ----
                for blk in range(NBLK):
                    xt = []
                    hi = 0
                    for k in range(KT):
                        t = xpool.tile([128, 512], XDT, tag=f"x{k}", name=f"x{k}")
                        for j in range(2):
                            n = 2 * blk + j
                            # x_split: one ACT half + one DVE half per tile
                            # (parallel engines -> lower tile-ready latency);
                            # else the first act_x halves go to ACT.
                            on_act = (j == 0) if x_split else (hi < act_x)
                            if on_act:
                                nc.scalar.activation(
                                    out=t[:, ts(j, NE)],
                                    in_=ep_t[k][:],
                                    func=relu,
                                    bias=hp_t[k][:, n : n + 1],
                                    scale=1.0,
                                )
                            else:
                                nc.vector.tensor_scalar(
                                    out=t[:, ts(j, NE)],
                                    in0=ep_t[k][:],
                                    scalar1=hp_t[k][:, n : n + 1],
                                    scalar2=0.0,
                                    op0=add,
                                    op1=amax,
                                )
                            hi += 1
                        xt.append(t)

                    for s in range(ST):
                        psy = y_ps.tile([128, 512], F32, tag="Y")
                        for k in range(KT):
                            nc.tensor.matmul(
                                psy[:],
                                xt[k][:, ts(s, 128)],
                                w2_t[k][:],
                                start=(k == 0),
                                stop=(k == KT - 1),
                            )
                        src = psy
                        if with_b2:
                            tt = tpool.tile([128, 512], BF16, tag=f"t{s}",
                                            name=f"t{s}")
                            nc.vector.tensor_add(out=tt[:], in0=psy[:],
                                                 in1=b2r_t[:])
                            src = tt
                        zt = zpool.tile([128, 512], BF16, tag="z", name="z")
                        nc.vector.scalar_tensor_tensor(
                            out=zt[:],
                            in0=src[:],
                            scalar=0.0,
                            in1=sgr_t[:],
                            op0=amax,
                            op1=mult,
                            accum_out=racc[:, 4 * blk + s : 4 * blk + s + 1],
                        )

                # ---- epilogue: transpose racc, add b3, one contiguous DMA ----
                pst = t_ps.tile([128, 128], F32, tag="pT", name="pT")
                nc.tensor.transpose(pst[:], racc[:], idn_t[:])
                ot = opool.tile([128, 128], F32, tag="o")
                nc.scalar.activation(
                    out=ot[:], in_=pst[:], func=ident,
                    bias=b3r_t[:, 0:1], scale=1.0,
                )
                nc.sync.dma_start(out=out_d[:], in_=ot[:])

            if dyn_repeat is not None:
                hint = (mybir.EngineType.PE, mybir.EngineType.DVE,
                        mybir.EngineType.Activation)
                if hint_sp:
                    hint = hint + (mybir.EngineType.SP,)
                # Unroll the hardware loop by the largest divisor of
                # dyn_repeat <= max_unroll: fewer For_i all-engine barriers,
                # and pool rotation double-buffers ep/hp/racc across the
                # unrolled bodies. Total body count stays exactly dyn_repeat.
                u = 1
                if unroll2:
                    for cand in (8, 4, 2):
                        if cand <= max_unroll and dyn_repeat % cand == 0:
                            u = cand
                            break
                with tc.For_i(0, dyn_repeat // u, 1, hint_engines=hint):
                    for _ in range(u):
                        body()
            else:
                for _rep in range(repeat):
                    body()
    return nc


def make_in_maps(h_all, e_feat, W1, b1, W2, b2, W3, b3):
    bf = ml_dtypes.bfloat16
    Wh = np.ascontiguousarray(W1[:H]).astype(bf).reshape(HT, 128, HID)
    We_aug = np.concatenate([W1[H:], b1[None, :]], axis=0).astype(bf)
    eT_aug = np.concatenate(
        [e_feat.T, np.ones((1, NE), np.float32)], axis=0
    ).astype(bf)
    w3 = np.asarray(W3, np.float32).reshape(HID)
    aw3 = np.abs(w3)
    sg3 = np.sign(w3).astype(np.float32)
    W2s = (np.asarray(W2, np.float32) * aw3[None, :])
    b2s = np.asarray(b2, np.float32) * aw3
    W2k = W2s.astype(bf).reshape(KT, 128, HID)
    b2r = np.broadcast_to(b2s[None, :], (128, HID)).astype(bf)
    sgr = np.broadcast_to(sg3[None, :], (128, HID)).astype(bf)
    b3r = np.full((128, 1), float(np.asarray(b3).reshape(-1)[0]), np.float32)
    idn = np.eye(128, dtype=np.float32)
    shared = {
        "wh": Wh, "we": We_aug, "et": eT_aug, "w2": W2k,
        "b2r": np.ascontiguousarray(b2r), "sgr": np.ascontiguousarray(sgr),
        "b3r": b3r, "idn": idn,
    }
    in_maps = []
    for b in range(B):
        hT = np.ascontiguousarray(h_all[b].T).astype(bf).reshape(HT, 128, N)
        in_maps.append({"ht": hT, **shared})
    return in_maps


_nc_cache = {}


def _get_nc(with_b2):
    key = ("nc", bool(with_b2))
    if key not in _nc_cache:
        nc = bacc.Bacc("TRN2", target_bir_lowering=False, debug=False, num_devices=B)
        build(nc, with_b2=with_b2)
        nc.compile()
        _nc_cache[key] = nc
    return _nc_cache[key]


def kernel(h_all, e_feat, W1, b1, W2, b2, W3, b3):
    h_all = np.asarray(h_all, np.float32)
    e_feat = np.asarray(e_feat, np.float32)
    W1 = np.asarray(W1, np.float32)
    b1 = np.asarray(b1, np.float32)
    W2 = np.asarray(W2, np.float32)
    b2 = np.asarray(b2, np.float32)
    W3 = np.asarray(W3, np.float32)
    b3 = np.asarray(b3, np.float32)

    # b2 == 0 makes the pre-relu bias add a no-op; select the graph that
    # skips that pass (the general graph handles nonzero b2).
    with_b2 = bool(np.any(b2 != 0))
    nc = _get_nc(with_b2)
    in_maps = make_in_maps(h_all, e_feat, W1, b1, W2, b2, W3, b3)
    res = run_bass_kernel_spmd(nc, in_maps, core_ids=list(range(B)))
    out = np.stack([res.results[i]["out"].reshape(N, NE, OUT) for i in range(B)])
    return out.astype(np.float32)


# revision 41
# speedup vs baseline: 1.0318x; 1.0184x over previous
"""Trainium2 Bass kernel for AllAtomEnergyBranch (3-layer MLP over broadcast concat).

Math (per batch b, position n, edge e):
    out[b,n,e,0] = W3^T relu(W2^T relu(Wh^T h[b,n] + We^T e_feat[e] + b1) + b2) + b3

Sharding: data-parallel over B (8 batches -> 8 NeuronCores), weights replicated.
Each core computes its [64, 256] output slice independently; no collectives.

v3 dataflow — transposed mm2 so the W3 contraction is a FREE-dim reduce:
  - |w3| is folded into W2 and b2 host-side (W2' = W2*|w3|, b2' = b2*|w3|),
    leaving only the sign vector sigma = sign(w3) for the reduce.
  - epT [512k, 256e] = (We_aug.T @ eT_aug) with b1 via augmented ones-row.
  - hpT [512k, 64n]  = Wh.T @ hT, f32.
  - per block of 2 n's (32 blocks):
      X^T[kt] [128,512] = relu(epT[kt] + hpT[kt][:,n])     (ACT, bias=hp col)
      per 128-col subtile s of the block:
        psY[s][128x,512j] = sum_kt X^T[kt][:,s].T @ W2'[kt]  (PE; X stationary,
                             W2' moving — j lives in the free dim now)
        t = psY + b2rep                                      (DVE tensor_tensor)
        z = relu(t) * sigmarep, racc[:, 4*blk+s] = sum_j z   (DVE scalar_tensor_tensor
                                                              with accum_out)
  - per iteration: PE-transpose racc [128x, 128p] -> [128p, 128x] (p = 4*blk+s
    indexes 128 consecutive outputs), ACT Identity+b3 copy to SBUF, one 64KB
    contiguous DMA ([128,128] f32 == row-major [64,256] output).

This removes the 32 serialized partition-reduce matmuls (~8us PE) and all
per-block output ops; PE does only mm2 + preamble + one transpose. The
benchmark For_i loop is 4x-unrolled with pool-rotated ep/hp/racc buffers:
fewer For_i all-engine barriers and the next iteration's preamble overlaps
the prior tail (~10us for 2x + ~4.6us more for 4x on HW; 8x regresses,
as does adding SP to hint_engines). act_x=4 (X-build split 4 ACT / 4 DVE)
beat act_x=5 by ~3us under the 4x unroll in interleaved A/B; successive
A/B rounds walked it monotonically down to act_x=0 (each step ~2us: the
DVE's 3x-faster per-op X-build beats ACT at every step; DVE tops out at
~117us, still under the PE roofline). Per-tile ACT/DVE
half-splitting measured slightly worse than block assignment. The preamble
interleaves (ep[k], hp[k]) pairs in k-order so the first X-build starts as
early as possible after the barrier (-2.1us with ehp/racc pools at bufs=4).
X-in-fp8-e3m4 (x_fp8 flag) halves LDWEIGHTS streaming and measured -3.6us
at rel err 0.0161 / absmax 0.0231 — left off by default (absmax crosses
2e-2 if the gate were interpreted that way).

Perf (measured on TRN2 silicon, For_i-loop delta-R method; chip-thermal
variance is several us between runs):
  - TimelineSim (no LDWEIGHTS model): 122.8us single shot; PE busy 110.0us
    (= bf16 streaming roofline for mm2 + preamble + transpose), DVE ~99us,
    ACT ~65us hidden underneath.
  - hardware steady state: 135-148us/iteration depending on thermal state
    (vs 153-180us for the previous per-block-reduce kernel on the same
    days' chip states; ~28us of the HW-sim gap is per-LDWEIGHTS exposure,
    toolchain-structural at ~53ns per matmul).
  - rel err vs f32 reference: 0.0039 (harness gate 2e-2).
"""

import numpy as np
import ml_dtypes

import concourse.bass as bass
import concourse.mybir as mybir
from concourse import bacc
from concourse.bass import ts
from concourse.tile import TileContext
from concourse.bass_utils import run_bass_kernel_spmd

BF16 = mybir.dt.bfloat16
F32 = mybir.dt.float32

B, N, H = 8, 64, 256
NE, E = 256, 64
HID, OUT = 512, 1
KT = HID // 128   # 4 k-tiles of layer-1 output / layer-2 contraction
JT = HID // 128   # 4 j-tiles of layer-2 output (free dim of mm2 now)
HT = H // 128     # 2 h-tiles of layer-1 contraction
NBLK = N // 2     # blocks of 2 n-values -> 512 x-columns per block
ST = 4            # 128-col stationary subtiles per block


def build(nc, repeat=1, dyn_repeat=None, y_bufs=7, x_bufs=3, t_bufs=3,
          with_b2=False, act_x=0, unroll2=True, pt_bufs=1, max_unroll=4,
          hint_sp=False, x_fp8=False, ehp_bufs=4, r_bufs=4,
          pre_ilv=True, x_split=False, act_copies=False, stag=False):
    """Build the per-core graph. All 8 cores run this same program.

    with_b2: include the per-free-element b2 bias add before the layer-2
    relu. When the runtime b2 vector is all zeros (as in the harness
    inputs), kernel() selects the with_b2=False graph, which skips that
    DVE pass entirely — the math is identical for b2 == 0.
    act_x: how many of the 8 X-build half-tiles per block run on ACT
    (the rest run on DVE) — engine load balancing knob.
    x_fp8: store X in fp8-e3m4 instead of bf16. FWL streams 4 fp8/cycle
    vs 2 bf16, halving the per-matmul LDWEIGHTS cost (~27ns vs ~53ns,
    ~13us/iter); the matmul itself runs at bf16 speed (no DoubleRow).
    Costs accuracy: rel err 0.016 vs 0.0039 (gate 2e-2) on the harness
    inputs — kernel() keeps bf16; opt-in for benchmarking.

    repeat / dyn_repeat: repeat the whole computation inside the NEFF
    (python-unrolled / For_i hardware loop) — used only for benchmarking.
    """
    ht_d = nc.declare_dram_parameter("ht", [HT, 128, N], BF16, isOutput=False)
    wh_d = nc.declare_dram_parameter("wh", [HT, 128, HID], BF16, isOutput=False)
    we_d = nc.declare_dram_parameter("we", [E + 1, HID], BF16, isOutput=False)
    et_d = nc.declare_dram_parameter("et", [E + 1, NE], BF16, isOutput=False)
    w2_d = nc.declare_dram_parameter("w2", [KT, 128, HID], BF16, isOutput=False)
    b2r_d = (nc.declare_dram_parameter("b2r", [128, HID], BF16, isOutput=False)
             if with_b2 else None)
    sgr_d = nc.declare_dram_parameter("sgr", [128, HID], BF16, isOutput=False)
    b3r_d = nc.declare_dram_parameter("b3r", [128, 1], F32, isOutput=False)
    idn_d = nc.declare_dram_parameter("idn", [128, 128], F32, isOutput=False)
    out_d = nc.declare_dram_parameter("out", [128, 128], F32, isOutput=True)

    XDT = mybir.dt.float8e3 if x_fp8 else BF16
    relu = mybir.ActivationFunctionType.Relu
    ident = mybir.ActivationFunctionType.Identity
    add = mybir.AluOpType.add
    mult = mybir.AluOpType.mult
    amax = mybir.AluOpType.max

    with TileContext(nc) as tc:
        with (
            tc.tile_pool(name="const", bufs=1) as cpool,
            tc.tile_pool(name="xp", bufs=x_bufs) as xpool,
            tc.tile_pool(name="tp", bufs=t_bufs) as tpool,
            tc.tile_pool(name="zp", bufs=4) as zpool,
            tc.tile_pool(name="rp", bufs=r_bufs) as rpool,
            tc.tile_pool(name="ehp", bufs=ehp_bufs) as eppool,
            tc.tile_pool(name="op", bufs=2) as opool,
            tc.tile_pool(name="psY", bufs=y_bufs, space="PSUM") as y_ps,
            tc.tile_pool(name="psT", bufs=pt_bufs, space="PSUM") as t_ps,
        ):
            # ---- load weights / inputs into SBUF ----
            # Order matters: everything the preamble matmuls need (ht/we/et/wh)
            # goes first so the PE can start while W2 is still in flight.
            we_t = cpool.tile([E + 1, HID], BF16, tag="we")
            nc.sync.dma_start(out=we_t[:], in_=we_d[:])
            et_t = cpool.tile([E + 1, NE], BF16, tag="et")
            nc.sync.dma_start(out=et_t[:], in_=et_d[:])
            ht_t = []
            for h in range(HT):
                t = cpool.tile([128, N], BF16, tag=f"ht{h}", name=f"ht{h}")
                nc.sync.dma_start(out=t[:], in_=ht_d[h])
                ht_t.append(t)
            wh_t = []
            for h in range(HT):
                t = cpool.tile([128, HID], BF16, tag=f"wh{h}", name=f"wh{h}")
                nc.sync.dma_start(out=t[:], in_=wh_d[h])
                wh_t.append(t)
            b2r_t = None
            if with_b2:
                b2r_t = cpool.tile([128, HID], BF16, tag="b2r")
                nc.sync.dma_start(out=b2r_t[:], in_=b2r_d[:])
            sgr_t = cpool.tile([128, HID], BF16, tag="sgr")
            nc.sync.dma_start(out=sgr_t[:], in_=sgr_d[:])
            b3r_t = cpool.tile([128, 1], F32, tag="b3r")
            nc.sync.dma_start(out=b3r_t[:], in_=b3r_d[:])
            idn_t = cpool.tile([128, 128], F32, tag="idn")
            nc.sync.dma_start(out=idn_t[:], in_=idn_d[:])
            w2_t = []
            for k in range(KT):
                t = cpool.tile([128, HID], BF16, tag=f"w2{k}", name=f"w2{k}")
                nc.sync.dma_start(out=t[:], in_=w2_d[k])
                w2_t.append(t)

            def body():
                # ---- preamble: epT (with b1 via aug row) and hpT ----
                # ep/hp tiles come from a bufs=2 pool and are allocated inside
                # body(), so a 2x-unrolled loop double-buffers them across
                # consecutive iterations (next preamble overlaps prior tail).
                ep_t = [eppool.tile([128, NE], BF16, tag=f"ep{k}", name=f"ep{k}")
                        for k in range(KT)]
                hp_t = [eppool.tile([128, N], F32, tag=f"hp{k}", name=f"hp{k}")
                        for k in range(KT)]
                # act_copies: the psum->SBUF preamble copies run on ACT
                # (idle at act_x=0) instead of DVE, where they would queue
                # behind the previous bodies' still-draining STT work right
                # at the loop boundary.
                def _copy(out, in_):
                    if act_copies:
                        nc.scalar.activation(out=out, in_=in_, func=ident,
                                             scale=1.0)
                    else:
                        nc.vector.tensor_copy(out=out, in_=in_)

                def emit_psE(k):
                    ps = y_ps.tile([128, NE], F32, tag="Y", name="psE")
                    nc.tensor.matmul(
                        ps[:], we_t[:, ts(k, 128)], et_t[:], start=True, stop=True
                    )
                    _copy(ep_t[k][:], ps[:])

                def emit_psH(k):
                    ps = y_ps.tile([128, N], F32, tag="Y", name="psH")
                    for h in range(HT):
                        nc.tensor.matmul(
                            ps[:],
                            wh_t[h][:, ts(k, 128)],
                            ht_t[h][:],
                            start=(h == 0),
                            stop=(h == HT - 1),
                        )
                    _copy(hp_t[k][:], ps[:])

                if pre_ilv:
                    # (ep[k], hp[k]) pairs complete in k-order so the first
                    # X-build can start as early as possible after the
                    # loop-back barrier.
                    for k in range(KT):
                        emit_psE(k)
                        emit_psH(k)
                else:
                    for k in range(KT):
                        emit_psE(k)
                    for k in range(KT):
                        emit_psH(k)

                # per-iteration accumulator of the 128 output rows
                racc = rpool.tile([128, 128], F32, tag="racc", name="racc")

                # ---- main loop over blocks of 2 n-values ----
                for blk in range(NBLK):
                    xt = []
                    hi = 0
                    for k in range(KT):
                        t = xpool.tile([128, 512], XDT, tag=f"x{k}", name=f"x{k}")
                        for j in range(2):
                            n = 2 * blk + j
                            # x_split: one ACT half + one DVE half per tile
                            # (parallel engines -> lower tile-ready latency);
                            # else the first act_x halves go to ACT.
                            on_act = (j == 0) if x_split else (hi < act_x)
                            if on_act:
                                nc.scalar.activation(
                                    out=t[:, ts(j, NE)],
                                    in_=ep_t[k][:],
                                    func=relu,
                                    bias=hp_t[k][:, n : n + 1],
                                    scale=1.0,
                                )
                            else:
                                nc.vector.tensor_scalar(
                                    out=t[:, ts(j, NE)],
                                    in0=ep_t[k][:],
                                    scalar1=hp_t[k][:, n : n + 1],
                                    scalar2=0.0,
                                    op0=add,
                                    op1=amax,
                                )
                            hi += 1
                        xt.append(t)

                    for s in range(ST):
                        psy = y_ps.tile([128, 512], F32, tag="Y")
                        for k in range(KT):
                            nc.tensor.matmul(
                                psy[:],
                                xt[k][:, ts(s, 128)],
                                w2_t[k][:],
                                start=(k == 0),
                                stop=(k == KT - 1),
                            )
                        src = psy
                        if with_b2:
                            tt = tpool.tile([128, 512], BF16, tag=f"t{s}",
                                            name=f"t{s}")
                            nc.vector.tensor_add(out=tt[:], in0=psy[:],
                                                 in1=b2r_t[:])
                            src = tt
                        zt = zpool.tile([128, 512], BF16, tag="z", name="z")
                        nc.vector.scalar_tensor_tensor(
                            out=zt[:],
                            in0=src[:],
                            scalar=0.0,
                            in1=sgr_t[:],
                            op0=amax,
                            op1=mult,
                            accum_out=racc[:, 4 * blk + s : 4 * blk + s + 1],
                        )

                # ---- epilogue: transpose racc, add b3, one contiguous DMA ----
                pst = t_ps.tile([128, 128], F32, tag="pT", name="pT")
                nc.tensor.transpose(pst[:], racc[:], idn_t[:])
                ot = opool.tile([128, 128], F32, tag="o")
                nc.scalar.activation(
                    out=ot[:], in_=pst[:], func=ident,
                    bias=b3r_t[:, 0:1], scale=1.0,
                )
                nc.sync.dma_start(out=out_d[:], in_=ot[:])

            if dyn_repeat is not None:
                hint = (mybir.EngineType.PE, mybir.EngineType.DVE,
                        mybir.EngineType.Activation)
                if hint_sp:
                    hint = hint + (mybir.EngineType.SP,)
                # Unroll the hardware loop by the largest divisor of
                # dyn_repeat <= max_unroll: fewer For_i all-engine barriers,
                # and pool rotation double-buffers ep/hp/racc across the
                # unrolled bodies. Total body count stays exactly dyn_repeat.
                u = 1
                if unroll2:
                    for cand in (8, 4, 2):
                        if cand <= max_unroll and dyn_repeat % cand == 0:
                            u = cand
                            break
                with tc.For_i(0, dyn_repeat // u, 1, hint_engines=hint,
                              staggered_reset=stag):
                    for _ in range(u):
                        body()
            else:
                for _rep in range(repeat):
                    body()
    return nc


def make_in_maps(h_all, e_feat, W1, b1, W2, b2, W3, b3):
    bf = ml_dtypes.bfloat16
    Wh = np.ascontiguousarray(W1[:H]).astype(bf).reshape(HT, 128, HID)
    We_aug = np.concatenate([W1[H:], b1[None, :]], axis=0).astype(bf)
    eT_aug = np.concatenate(
        [e_feat.T, np.ones((1, NE), np.float32)], axis=0
    ).astype(bf)
    w3 = np.asarray(W3, np.float32).reshape(HID)
    aw3 = np.abs(w3)
    sg3 = np.sign(w3).astype(np.float32)
    W2s = (np.asarray(W2, np.float32) * aw3[None, :])
    b2s = np.asarray(b2, np.float32) * aw3
    W2k = W2s.astype(bf).reshape(KT, 128, HID)
    b2r = np.broadcast_to(b2s[None, :], (128, HID)).astype(bf)
    sgr = np.broadcast_to(sg3[None, :], (128, HID)).astype(bf)
    b3r = np.full((128, 1), float(np.asarray(b3).reshape(-1)[0]), np.float32)
    idn = np.eye(128, dtype=np.float32)
    shared = {
        "wh": Wh, "we": We_aug, "et": eT_aug, "w2": W2k,
        "b2r": np.ascontiguousarray(b2r), "sgr": np.ascontiguousarray(sgr),
        "b3r": b3r, "idn": idn,
    }
    in_maps = []
    for b in range(B):
        hT = np.ascontiguousarray(h_all[b].T).astype(bf).reshape(HT, 128, N)
        in_maps.append({"ht": hT, **shared})
    return in_maps


_nc_cache = {}


def _get_nc(with_b2):
    key = ("nc", bool(with_b2))
    if key not in _nc_cache:
        nc = bacc.Bacc("TRN2", target_bir_lowering=False, debug=False, num_devices=B)
        build(nc, with_b2=with_b2)
        nc.compile()
        _nc_cache[key] = nc
    return _nc_cache[key]


def kernel(h_all, e_feat, W1, b1, W2, b2, W3, b3):
    h_all = np.asarray(h_all, np.float32)
    e_feat = np.asarray(e_feat, np.float32)
    W1 = np.asarray(W1, np.float32)
    b1 = np.asarray(b1, np.float32)
    W2 = np.asarray(W2, np.float32)
    b2 = np.asarray(b2, np.float32)
    W3 = np.asarray(W3, np.float32)
    b3 = np.asarray(b3, np.float32)

    # b2 == 0 makes the pre-relu bias add a no-op; select the graph that
    # skips that pass (the general graph handles nonzero b2).
    with_b2 = bool(np.any(b2 != 0))
    nc = _get_nc(with_b2)
    in_maps = make_in_maps(h_all, e_feat, W1, b1, W2, b2, W3, b3)
    res = run_bass_kernel_spmd(nc, in_maps, core_ids=list(range(B)))
    out = np.stack([res.results[i]["out"].reshape(N, NE, OUT) for i in range(B)])
    return out.astype(np.float32)
